# revision 11
# baseline (speedup 1.0000x reference)
"""Trainium2 Bass kernel for MoD (mixture-of-depths) routing FFN.

Semantics (matching the reference):
  w = x @ W_r + b_r                        # [B, S] router weights
  t_b = K-th largest of w[b, :]            # per-row threshold (K=512)
  selected: w > t_b (strict; ties at threshold dropped)
  out[b, s] = w[b,s] * (gelu(x[b,s] @ W1 + b1) @ W2 + b2)   if selected
  out[b, s] = x[b, s]                                        otherwise

Sharding: 8 cores; cores (2b, 2b+1) form a pair handling batch row b.
Each core routes half the row; the pair AllGathers router weights; exact
top-K selection runs via a 128-bin histogram (exact compares only inside
the bin containing the K-th value); selected tokens are compacted into K
slots via matmul-based stream compaction; the FFN runs tensor-parallel
over the pair (W1 column-split / W2 row-split, bf16 compute); partial
outputs are gate-scaled, exchanged pairwise via bf16 AllGather chunks and
summed locally before the scatter. Routing, selection and the residual
path stay fully fp32.
"""

from contextlib import ExitStack

import numpy as np

import concourse.bass as bass
import concourse.tile as tile
from concourse import mybir
from concourse.bass import IndirectOffsetOnAxis
from concourse.bass_utils import run_bass_kernel_spmd
from concourse.masks import make_identity
from concourse.tile_rust import add_dep_helper

F32 = mybir.dt.float32
BF16 = mybir.dt.bfloat16
FP16 = mybir.dt.float16
I32 = mybir.dt.int32

NC_CORES = 8
DEBUG_DUMPS = False

NBINS = 128          # histogram bins over w in [-4, 4]
BIN_LO = -4.0
BIN_SCALE = NBINS / 8.0


def build_mod_kernel(nc, S, D, DFF, K):
    """Emit the per-core SPMD program. Pair = (2b, 2b+1) handles row b.

    Inputs (per-core):
      x_own  [S/2, D] f32   this core's half-row (router + residual src)
      x_bf   [S, D]   bf16  the full row in bf16 (gather source for FFN)
      wr     [1, D]   f32   router weight
      br     [1, 1]   f32   router bias
      w1     [4, ND, 128, 1024] bf16  W1 column shard, mp-blocked
      w2     [2, NM, 128, 1024] bf16  W2 row shard, d-group blocked
      b1s    [128, NM] f32  b1 shard (pre-transposed)
      b2h    [1, D]   bf16  0.5 * b2
      hoff   [1, 1]   f32   h * S/2  (0 for even cores, S/2 for odd)
    Output:
      out    [S/2, D] f32
    """
    HALF = S // 2
    DFFH = DFF // 2
    CAP = K                      # slots per row (max selected = K-1 < CAP)
    KT = HALF // 128             # x tiles per core (16)
    TT = S // 128                # token tiles per row (32)
    NJ = CAP // 128              # slot tiles (4)
    ND = D // 128                # d 128-tiles (16)
    NM = DFFH // 128             # dff-col tiles (32)
    NMP = NM // 8                # mm1 mp-groups of 8 m-tiles (4)
    NGRP = D // 512              # mm2 d-chunks == exchange chunks (4)

    x_own = nc.declare_dram_parameter("x_own", [HALF, D], F32, isOutput=False)
    x_bf = nc.declare_dram_parameter("x_bf", [S, D], BF16, isOutput=False)
    wr = nc.declare_dram_parameter("wr", [1, D], F32, isOutput=False)
    br = nc.declare_dram_parameter("br", [1, 1], F32, isOutput=False)
    w1 = nc.declare_dram_parameter("w1", [NMP, ND, 128, 1024], BF16,
                                   isOutput=False)
    w2 = nc.declare_dram_parameter("w2", [2, NM, 128, 1024], BF16,
                                   isOutput=False)
    b1s = nc.declare_dram_parameter("b1s", [128, NM], F32, isOutput=False)
    b2h = nc.declare_dram_parameter("b2h", [1, D], BF16, isOutput=False)
    hoff = nc.declare_dram_parameter("hoff", [1, 1], F32, isOutput=False)
    out = nc.declare_dram_parameter("out", [HALF, D], F32, isOutput=True)

    # Internal DRAM for collectives (pair groups).
    ag_in = nc.dram_tensor("ag_in", [1, HALF], F32)
    ag_out = nc.dram_tensor("ag_out", [2, HALF], F32)
    ex_in = nc.dram_tensor("ex_in", [NGRP, CAP, 512], BF16)
    ex_out = nc.dram_tensor("ex_out", [NGRP, 2, CAP, 512], BF16)
    pairs = [[2 * b, 2 * b + 1] for b in range(NC_CORES // 2)]

    with tile.TileContext(nc) as tc, ExitStack() as ctx:
        pc = ctx.enter_context(tc.tile_pool(name="const", bufs=1))
        pr = ctx.enter_context(tc.tile_pool(name="route", bufs=1))

        # ---- small input broadcasts (Pool queue; keep this list short) ----
        wr1 = pc.tile([1, D], F32, name="wr1")
        nc.sync.dma_start(wr1[:], wr.ap())
        wr_bc = pc.tile([128, D], F32, name="wr_bc")
        nc.gpsimd.partition_broadcast(wr_bc[:], wr1[:], 128)
        br1 = pc.tile([1, 1], F32, name="br1")
        nc.sync.dma_start(br1[:], br.ap())
        br_bc = pc.tile([128, 1], F32, name="br_bc")
        nc.gpsimd.partition_broadcast(br_bc[:], br1[:], 128)
        ho1 = pc.tile([1, 1], F32, name="ho1")
        nc.sync.dma_start(ho1[:], hoff.ap())
        ho_bc = pc.tile([128, 1], F32, name="ho_bc")
        nc.gpsimd.partition_broadcast(ho_bc[:], ho1[:], 128)
        # b1_sb[p, m] = b1[m*128 + p] (host pre-transposed)
        b1_sb = pc.tile([128, NM], F32, name="b1_sb")
        nc.scalar.dma_start(b1_sb[:], b1s.ap())
        b2_sb = pc.tile([1, D], BF16, name="b2_sb")
        nc.scalar.dma_start(b2_sb[:], b2h.ap())

        # ---- constants ----
        ident = pc.tile([128, 128], F32, name="ident")
        make_identity(nc, ident[:])
        ident_b = pc.tile([128, 128], BF16, name="ident_b")
        nc.vector.tensor_copy(ident_b[:], ident[:])
        ones128 = pc.tile([128, 1], F32, name="ones128")
        nc.vector.memset(ones128[:], 1.0)
        ones1b = pc.tile([1, 128], BF16, name="ones1b")
        nc.vector.memset(ones1b[:], 1.0)
        onesb_col = pc.tile([128, 1], BF16, name="onesb_col")
        nc.vector.memset(onesb_col[:], 1.0)
        # U strict-upper triangulars (as stored): U[q, p] = 1 iff q < p
        uTT = pc.tile([TT, TT], F32, name="uTT")
        nc.gpsimd.memset(uTT[:], 0.0)
        nc.gpsimd.affine_select(
            out=uTT[:], in_=uTT[:], compare_op=mybir.AluOpType.is_ge,
            fill=1.0, base=0, pattern=[[-1, TT]], channel_multiplier=1,
        )
        u128 = pc.tile([128, 128], F32, name="u128")
        nc.gpsimd.memset(u128[:], 0.0)
        nc.gpsimd.affine_select(
            out=u128[:], in_=u128[:], compare_op=mybir.AluOpType.is_ge,
            fill=1.0, base=0, pattern=[[-1, 128]], channel_multiplier=1,
        )
        # slot iotas: f32 (bin compact + bins) and fp16 (slot one-hot)
        s_iota = pc.tile([128, CAP], F32, name="s_iota")
        nc.gpsimd.iota(s_iota[:], pattern=[[1, CAP]], base=0,
                       channel_multiplier=0, allow_small_or_imprecise_dtypes=True)
        s_iota_h = pc.tile([128, CAP], FP16, name="s_iota_h")
        nc.gpsimd.iota(s_iota_h[:], pattern=[[1, CAP]], base=0,
                       channel_multiplier=0, allow_small_or_imprecise_dtypes=True)
        # iota over partitions [128, 1]: value = p
        iota_p = pc.tile([128, 1], F32, name="iota_p")
        nc.gpsimd.iota(iota_p[:], pattern=[[0, 1]], base=0,
                       channel_multiplier=1, allow_small_or_imprecise_dtypes=True)
        # compact lhsT rows, fp16-exact: [p+1, c, gate] per token column c
        tg3 = pc.tile([128, 3 * TT], FP16, name="tg3")
        tg3v = tg3[:].rearrange("p (c three) -> p c three", three=3)
        nc.gpsimd.iota(tg3v[:, :, 0], pattern=[[0, TT]], base=1,
                       channel_multiplier=1, allow_small_or_imprecise_dtypes=True)
        nc.gpsimd.iota(tg3v[:, :, 1], pattern=[[1, TT]], base=0,
                       channel_multiplier=0, allow_small_or_imprecise_dtypes=True)

        # ---- phase R: router dot ----
        w_mine = pr.tile([128, KT], F32, name="w_mine")
        with tc.tile_pool(name="xs", bufs=6) as px, \
             tc.tile_pool(name="jr", bufs=1) as pjr:
            for k in range(KT):
                xt = px.tile([128, D], F32)
                eng = nc.sync if (k % 2 == 0) else nc.scalar
                eng.dma_start(xt[:], x_own.ap()[k * 128:(k + 1) * 128, :])
                jt = pjr.tile([128, D], F32, tag="jR")
                nc.vector.scalar_tensor_tensor(
                    out=jt[:], in0=xt[:], scalar=1.0, in1=wr_bc[:],
                    op0=mybir.AluOpType.bypass, op1=mybir.AluOpType.mult,
                    accum_out=w_mine[:, k:k + 1],
                )
            w_full = pr.tile([128, KT], F32, name="w_full")
            nc.vector.tensor_scalar_add(w_full[:], w_mine[:], br_bc[:, 0:1])
            # transpose to [KT, 128] so the DRAM write (l = k*128 + p) is
            # contiguous instead of a 4-byte-packet strided DMA
            with tc.tile_pool(name="pwt", bufs=1, space="PSUM") as pwt:
                wfT_ps = pwt.tile([KT, 128], F32, name="wfT_ps")
                nc.tensor.transpose(wfT_ps[:], w_full[:], ident[:])
                wfT = pr.tile([KT, 128], F32, name="wfT")
                nc.vector.tensor_copy(wfT[:], wfT_ps[:])
            nc.scalar.dma_start(
                ag_in.ap().rearrange("o (k p) -> (o k) p", p=128), wfT[:])

        # ---- AllGather router weights within pair ----
        ag_cc = nc.gpsimd.collective_compute(
            "AllGather", mybir.AluOpType.bypass, replica_groups=pairs,
            ins=[ag_in.ap()], outs=[ag_out.ap()],
        )

        # ---- phase RANK: exact top-K via 128-bin histogram ----
        # w_tok[p, c] = w[c*128 + p] over the full row, via two transposes
        # of the AllGather result (rank order in ag_out is [even, odd] =
        # [first half, second half] on both cores).
        w_tok = pr.tile([128, TT], F32, name="w_tok")
        with tc.tile_pool(name="pagt", bufs=2) as pagt, \
             tc.tile_pool(name="pwk", bufs=2, space="PSUM") as pwk:
            for h in range(2):
                agh = pagt.tile([KT, 128], F32, tag="agh")
                nc.scalar.dma_start(
                    agh[:],
                    ag_out.ap()[h:h + 1, :].rearrange(
                        "o (k p) -> (o k) p", p=128))
                wt_ps = pwk.tile([128, KT], F32, tag="wtps")
                nc.tensor.transpose(wt_ps[:], agh[:], ident[0:KT, 0:KT])
                nc.vector.tensor_copy(w_tok[:, h * KT:(h + 1) * KT], wt_ps[:])

        # bins: bin(t) = clamp(round_or_floor((w+4)*16), 0, 127), exact ints
        binf0 = pr.tile([128, TT], F32, name="binf0")
        nc.vector.tensor_scalar(
            out=binf0[:], in0=w_tok[:], scalar1=-BIN_LO, scalar2=BIN_SCALE,
            op0=mybir.AluOpType.add, op1=mybir.AluOpType.mult)
        binf1 = pr.tile([128, TT], F32, name="binf1")
        nc.vector.tensor_scalar(
            out=binf1[:], in0=binf0[:], scalar1=0.0, scalar2=float(NBINS - 1),
            op0=mybir.AluOpType.max, op1=mybir.AluOpType.min)
        bin_i = pr.tile([128, TT], I32, name="bin_i")
        nc.vector.tensor_copy(bin_i[:], binf1[:])
        bin_f = pr.tile([128, TT], F32, name="bin_f")
        nc.vector.tensor_copy(bin_f[:], bin_i[:])

        # histogram: hist[b] = #{t: bin(t) == b}, via one-hot matmuls
        hist = pr.tile([128, 1], F32, name="hist")
        nlt = pr.tile([128, 1], F32, name="nlt")
        with tc.tile_pool(name="poh", bufs=3) as poh, \
             tc.tile_pool(name="phs", bufs=2, space="PSUM") as phs:
            hist_ps = phs.tile([128, 1], F32, name="hist_ps")
            for c in range(TT):
                oh = poh.tile([128, NBINS], BF16, tag="oh")
                nc.vector.tensor_scalar(
                    out=oh[:], in0=s_iota[:, 0:NBINS],
                    scalar1=bin_f[:, c:c + 1], scalar2=None,
                    op0=mybir.AluOpType.is_equal)
                nc.tensor.matmul(hist_ps[:], lhsT=oh[:], rhs=onesb_col[:],
                                 start=(c == 0), stop=(c == TT - 1))
            nc.vector.tensor_copy(hist[:], hist_ps[:])
            # nlt[b] = #{t: bin(t) < b}
            nlt_ps = phs.tile([128, 1], F32, name="nlt_ps")
            nc.tensor.matmul(nlt_ps[:], lhsT=u128[:], rhs=hist[:],
                             start=True, stop=True)
            nc.vector.tensor_copy(nlt[:], nlt_ps[:])
        # ngt[b] = S - nlt[b] - hist[b]  (# tokens in bins above b)
        ngt = pr.tile([128, 1], F32, name="ngt")
        nc.vector.scalar_tensor_tensor(
            out=ngt[:], in0=nlt[:], scalar=-1.0, in1=hist[:],
            op0=mybir.AluOpType.mult, op1=mybir.AluOpType.subtract)
        nc.vector.tensor_scalar_add(ngt[:], ngt[:], float(S))
        # crossing bin b*: ngt(b*) <= K-1 and nlt(b*) <= S-K
        ind1 = pr.tile([128, 1], F32, name="ind1")
        nc.vector.tensor_scalar(out=ind1[:], in0=ngt[:], scalar1=float(K - 1),
                                scalar2=None, op0=mybir.AluOpType.is_le)
        ind2 = pr.tile([128, 1], F32, name="ind2")
        nc.vector.tensor_scalar(out=ind2[:], in0=nlt[:], scalar1=float(S - K),
                                scalar2=None, op0=mybir.AluOpType.is_le)
        ind = pr.tile([128, 1], F32, name="ind")
        nc.vector.tensor_tensor(out=ind[:], in0=ind1[:], in1=ind2[:],
                                op=mybir.AluOpType.mult)
        # extract [b*, ngt(b*)] to all partitions
        rhs3 = pr.tile([128, 2], F32, name="rhs3")
        nc.vector.tensor_copy(rhs3[:, 0:1], iota_p[:])
        nc.vector.tensor_copy(rhs3[:, 1:2], ngt[:])
        bst = pr.tile([1, 2], F32, name="bst")
        with tc.tile_pool(name="pbs3", bufs=1, space="PSUM") as pbs3:
            bst_ps = pbs3.tile([1, 2], F32, name="bst_ps")
            nc.tensor.matmul(bst_ps[:], lhsT=ind[:], rhs=rhs3[:],
                             start=True, stop=True)
            nc.vector.tensor_copy(bst[:], bst_ps[:])
        bst_bc = pr.tile([128, 2], F32, name="bst_bc")
        nc.gpsimd.partition_broadcast(bst_bc[:], bst[:], 128)

        # above / in-bin masks
        above = pr.tile([128, TT], F32, name="above")
        nc.vector.tensor_scalar(out=above[:], in0=bin_f[:],
                                scalar1=bst_bc[:, 0:1], scalar2=None,
                                op0=mybir.AluOpType.is_gt)
        inbin = pr.tile([128, TT], F32, name="inbin")
        nc.vector.tensor_scalar(out=inbin[:], in0=bin_f[:],
                                scalar1=bst_bc[:, 0:1], scalar2=None,
                                op0=mybir.AluOpType.is_equal)

        # compact the in-bin w values into wbin[1, 128] (slot order)
        with tc.tile_pool(name="pmc", bufs=3) as pmc, \
             tc.tile_pool(name="pmps", bufs=2, space="PSUM") as pmps:
            colTm_ps = pmps.tile([TT, 1], F32, name="colTm_ps")
            nc.tensor.matmul(colTm_ps[:], lhsT=inbin[:], rhs=ones128[:],
                             start=True, stop=True)
            colTm = pr.tile([TT, 1], F32, name="colTm")
            nc.vector.tensor_copy(colTm[:], colTm_ps[:])
            posm_ps = pmps.tile([128, TT], F32, name="posm_ps")
            nc.tensor.matmul(posm_ps[:], lhsT=colTm[:].to_broadcast([TT, 128]),
                             rhs=uTT[:], start=True, stop=False)
            nc.tensor.matmul(posm_ps[:], lhsT=u128[:], rhs=inbin[:],
                             start=False, stop=True)
            posm = pr.tile([128, TT], F32, name="posm")
            nc.vector.tensor_copy(posm[:], posm_ps[:])
            # not-in-bin -> push slot out of range:
            # posm_m = posm + 600*(1 - inbin) = posm - 600*inbin + 600
            posm_m = pr.tile([128, TT], F32, name="posm_m")
            nc.vector.scalar_tensor_tensor(
                out=posm_m[:], in0=inbin[:], scalar=-600.0, in1=posm[:],
                op0=mybir.AluOpType.mult, op1=mybir.AluOpType.add)
            nc.vector.tensor_scalar_add(posm_m[:], posm_m[:], 600.0)
            wsel_ps = pmps.tile([1, NBINS], F32, name="wsel_ps")
            for c in range(TT):
                eng = nc.vector if c % 2 == 0 else nc.gpsimd
                ohm = pmc.tile([128, NBINS], F32, tag="ohm")
                eng.tensor_scalar(
                    out=ohm[:], in0=s_iota[:, 0:NBINS],
                    scalar1=posm_m[:, c:c + 1], scalar2=None,
                    op0=mybir.AluOpType.is_equal)
                nc.tensor.matmul(wsel_ps[:], lhsT=w_tok[:, c:c + 1], rhs=ohm[:],
                                 start=(c == 0), stop=(c == TT - 1))
            wsel = pr.tile([1, NBINS], F32, name="wsel")
            nc.vector.tensor_copy(wsel[:], wsel_ps[:])
        wbin_bc = pr.tile([128, NBINS], F32, name="wbin_bc")
        nc.gpsimd.partition_broadcast(wbin_bc[:], wsel[:], 128)

        # counts_in(t) = #{j in b*: w_j >= w_t}; exact fp32 compares
        cin = pr.tile([128, TT], F32, name="cin")
        with tc.tile_pool(name="pci", bufs=2) as pci:
            for c in range(TT):
                jt = pci.tile([128, NBINS], F32, tag="jci")
                nc.vector.tensor_scalar(
                    out=jt[:], in0=wbin_bc[:], scalar1=w_tok[:, c:c + 1],
                    scalar2=None, op0=mybir.AluOpType.is_ge,
                    op1=mybir.AluOpType.add, accum_out=cin[:, c:c + 1])
        # selected = above OR (inbin AND cin + ngt(b*) <= K-1)
        cin2 = pr.tile([128, TT], F32, name="cin2")
        nc.vector.tensor_scalar(out=cin2[:], in0=cin[:],
                                scalar1=bst_bc[:, 1:2], scalar2=None,
                                op0=mybir.AluOpType.add)
        selin = pr.tile([128, TT], F32, name="selin")
        nc.vector.tensor_scalar(out=selin[:], in0=cin2[:],
                                scalar1=float(K - 1), scalar2=None,
                                op0=mybir.AluOpType.is_le)
        sel = pr.tile([128, TT], F32, name="sel")
        nc.vector.tensor_tensor(out=sel[:], in0=selin[:], in1=inbin[:],
                                op=mybir.AluOpType.mult)
        nc.vector.tensor_tensor(out=sel[:], in0=sel[:], in1=above[:],
                                op=mybir.AluOpType.add)
        unsel = pr.tile([128, TT], F32, name="unsel")
        nc.vector.tensor_scalar(out=unsel[:], in0=sel[:], scalar1=-1.0,
                                scalar2=1.0, op0=mybir.AluOpType.mult,
                                op1=mybir.AluOpType.add)
        gate = pr.tile([128, TT], F32, name="gate")
        nc.vector.tensor_tensor(out=gate[:], in0=sel[:], in1=w_tok[:],
                                op=mybir.AluOpType.mult)
        nc.vector.tensor_copy(tg3v[:, :, 2], gate[:])

        # ---- phase PREFIX: exclusive prefix-sum of sel over t = c*128+p ----
        with tc.tile_pool(name="pps", bufs=1, space="PSUM") as pps:
            colT_ps = pps.tile([TT, 1], F32, name="colT_ps")
            nc.tensor.matmul(colT_ps[:], lhsT=sel[:], rhs=ones128[:],
                             start=True, stop=True)
            colT = pr.tile([TT, 1], F32, name="colT")
            nc.vector.tensor_copy(colT[:], colT_ps[:])
            pos_ps = pps.tile([128, TT], F32, name="pos_ps")
            nc.tensor.matmul(pos_ps[:], lhsT=colT[:].to_broadcast([TT, 128]),
                             rhs=uTT[:], start=True, stop=False)
            nc.tensor.matmul(pos_ps[:], lhsT=u128[:], rhs=sel[:],
                             start=False, stop=True)
            pos = pr.tile([128, TT], F32, name="pos")
            nc.vector.tensor_copy(pos[:], pos_ps[:])
        # unselected -> slot 600+pos (never matches s_iota < CAP=512... use
        # 600 + pos <= 1111, exact in fp16, > 511)
        pos_m = pr.tile([128, TT], F32, name="pos_m")
        nc.vector.scalar_tensor_tensor(
            out=pos_m[:], in0=unsel[:], scalar=600.0, in1=pos[:],
            op0=mybir.AluOpType.mult, op1=mybir.AluOpType.add,
        )

        # ---- phase COMPACT: slot -> (p+1, c, gate) via fp16 matmuls ----
        tok_i = []   # int32 gather offsets per slot tile
        gate_s = []  # f32 per-slot gates
        dest_i = []  # int32 scatter offsets (OOB for pad/other-half)
        with tc.tile_pool(name="pcm", bufs=1, space="PSUM") as pcm, \
             tc.tile_pool(name="pmm", bufs=3) as pmm, \
             tc.tile_pool(name="ptp", bufs=4, space="PSUM") as ptp:
            cps = pcm.tile([3, CAP], F32, name="cps")
            for c in range(TT):
                eng = nc.vector if c % 2 == 0 else nc.gpsimd
                mt = pmm.tile([128, CAP], FP16, tag="mt")
                eng.tensor_scalar(
                    out=mt[:], in0=s_iota_h[:], scalar1=pos_m[:, c:c + 1],
                    scalar2=None, op0=mybir.AluOpType.is_equal,
                )
                nc.tensor.matmul(cps[:], lhsT=tg3[:, 3 * c:3 * c + 3], rhs=mt[:],
                                 start=(c == 0), stop=(c == TT - 1))
            compact = pr.tile([3, CAP], F32, name="compact")
            nc.vector.tensor_copy(compact[:], cps[:])
            for j in range(NJ):
                tp = ptp.tile([128, 3], F32, tag="tp")
                nc.tensor.transpose(tp[:], compact[:, j * 128:(j + 1) * 128],
                                    ident[0:3, 0:3])
                cpj = pr.tile([128, 3], F32, name=f"cpj{j}")
                nc.vector.tensor_copy(cpj[:], tp[:])
                gate_s.append(cpj)
                # tokp1 = 128*c + (p+1)  == token id + 1; 0 for pad slots
                tokp1 = pr.tile([128, 1], F32, name=f"tokp1{j}")
                nc.vector.scalar_tensor_tensor(
                    out=tokp1[:], in0=cpj[:, 1:2], scalar=128.0, in1=cpj[:, 0:1],
                    op0=mybir.AluOpType.mult, op1=mybir.AluOpType.add)
                # gather offset: max(tokp1 - 1, 0) -> int
                tif = pr.tile([128, 1], F32, name=f"tif{j}")
                nc.vector.tensor_scalar(
                    out=tif[:], in0=tokp1[:], scalar1=-1.0, scalar2=0.0,
                    op0=mybir.AluOpType.add, op1=mybir.AluOpType.max,
                )
                tii = pr.tile([128, 1], I32, name=f"tii{j}")
                nc.vector.tensor_copy(tii[:], tif[:])
                tok_i.append(tii)
                # scatter offset: (tokp1 - 1) - hoff, OOB for pad/other-half
                df = pr.tile([128, 1], F32, name=f"df{j}")
                nc.vector.scalar_tensor_tensor(
                    out=df[:], in0=tokp1[:], scalar=-1.0, in1=ho_bc[:],
                    op0=mybir.AluOpType.add, op1=mybir.AluOpType.subtract,
                )
                ok1 = pr.tile([128, 1], F32, name=f"ok1{j}")
                nc.vector.tensor_scalar(out=ok1[:], in0=df[:], scalar1=0.0,
                                        scalar2=None, op0=mybir.AluOpType.is_ge)
                ok2 = pr.tile([128, 1], F32, name=f"ok2{j}")
                nc.vector.tensor_scalar(out=ok2[:], in0=df[:],
                                        scalar1=float(HALF - 1), scalar2=None,
                                        op0=mybir.AluOpType.is_le)
                okm = pr.tile([128, 1], F32, name=f"okm{j}")
                nc.vector.tensor_tensor(out=okm[:], in0=ok1[:], in1=ok2[:],
                                        op=mybir.AluOpType.mult)
                # dfm = okm * (df - BIG) + BIG  (df when ok, BIG when not)
                BIG = float(8 * HALF + 11)
                dfs = pr.tile([128, 1], F32, name=f"dfs{j}")
                nc.vector.tensor_scalar_add(dfs[:], df[:], -BIG)
                dfm = pr.tile([128, 1], F32, name=f"dfm{j}")
                nc.vector.scalar_tensor_tensor(
                    out=dfm[:], in0=okm[:], scalar=BIG, in1=dfs[:],
                    op0=mybir.AluOpType.bypass, op1=mybir.AluOpType.mult)
                nc.vector.tensor_scalar_add(dfm[:], dfm[:], BIG)
                dii = pr.tile([128, 1], I32, name=f"dii{j}")
                nc.vector.tensor_copy(dii[:], dfm[:])
                dest_i.append(dii)

        # ---- phase GATHER: xg rows (bf16) -> transpose -> xgT ----
        xgT = pr.tile([128, ND, CAP], BF16, name="xgT")
        with tc.tile_pool(name="pxg", bufs=2) as pxg, \
             tc.tile_pool(name="ptg", bufs=4, space="PSUM") as ptg:
            for j in range(NJ):
                xg = pxg.tile([128, D], BF16, tag="xg")
                nc.gpsimd.indirect_dma_start(
                    out=xg[:], out_offset=None, in_=x_bf.ap(),
                    in_offset=IndirectOffsetOnAxis(ap=tok_i[j][:, 0:1], axis=0),
                )
                for kq in range(ND // 4):
                    tps = ptg.tile([128, 512], BF16, tag="tps")
                    for q in range(4):
                        k = kq * 4 + q
                        nc.tensor.transpose(
                            tps[:, q * 128:(q + 1) * 128],
                            xg[:, k * 128:(k + 1) * 128], ident_b[:])
                    eng = nc.vector if kq % 2 == 0 else nc.scalar
                    if eng is nc.vector:
                        for q in range(4):
                            k = kq * 4 + q
                            nc.vector.tensor_copy(
                                xgT[:, k, j * 128:(j + 1) * 128],
                                tps[:, q * 128:(q + 1) * 128])
                    else:
                        for q in range(4):
                            k = kq * 4 + q
                            nc.scalar.activation(
                                out=xgT[:, k, j * 128:(j + 1) * 128],
                                in_=tps[:, q * 128:(q + 1) * 128],
                                func=mybir.ActivationFunctionType.Copy,
                                scale=1.0)

        # ---- deferred residual copy: out = x (DRAM->DRAM), during MM1 ----
        residual_dmas = []
        for k in range(KT // 4):
            r = nc.sync.dma_start(
                out.ap()[k * 512:(k + 1) * 512, :],
                x_own.ap()[k * 512:(k + 1) * 512, :])
            residual_dmas.append(r)

        # ---- phase MM1 + gelu: h[dffcol, toks] = gelu(xg @ W1 + b1) ----
        h_all = pr.tile([128, NM, CAP], BF16, name="h_all")
        with tc.tile_pool(name="pw1", bufs=36) as pw1, \
             tc.tile_pool(name="ph1", bufs=1, space="PSUM") as ph1:
            for mp in range(NMP):
                hps = [ph1.tile([128, CAP], F32, tag=f"hp{i}", name=f"hp{i}")
                       for i in range(8)]
                for k in range(ND):
                    w1c = pw1.tile([128, 1024], BF16, tag="w1c")
                    nc.sync.dma_start(w1c[:], w1.ap()[mp, k])
                    for i in range(8):
                        nc.tensor.matmul(
                            hps[i][:], lhsT=w1c[:, i * 128:(i + 1) * 128],
                            rhs=xgT[:, k, :], start=(k == 0), stop=(k == ND - 1))
                for i in range(8):
                    m = mp * 8 + i
                    nc.scalar.activation(
                        out=h_all[:, m, :], in_=hps[i][:],
                        func=mybir.ActivationFunctionType.Gelu_apprx_tanh,
                        bias=b1_sb[:, m:m + 1], scale=1.0)

        # ---- phase MM2 + pair exchange + combine/scatter ----
        # blk_part[tok, d] = h.T @ W2 + 0.5*b2, gate-scaled, exchanged in
        # bf16 chunks of 512 d-cols; both halves summed locally.
        with tc.tile_pool(name="pw2", bufs=1) as pw2, \
             tc.tile_pool(name="pb2", bufs=2, space="PSUM") as pb2, \
             tc.tile_pool(name="pbs", bufs=6) as pbs, \
             tc.tile_pool(name="pfa", bufs=3) as pfa:
            w2t = [None] * NM
            for g in range(NGRP):
                gp, gh = g // 2, g % 2
                bps = [pb2.tile([128, 512], F32, tag=f"bp{j}", name=f"bp{j}")
                       for j in range(NJ)]
                for m in range(NM):
                    if gh == 0:
                        w2t[m] = pw2.tile([128, 1024], BF16, tag=f"w2c{m}", name=f"w2t{m}")
                        nc.sync.dma_start(w2t[m][:], w2.ap()[gp, m])
                    rhs = w2t[m][:, gh * 512:(gh + 1) * 512]
                    for j in range(NJ):
                        nc.tensor.matmul(
                            bps[j][:],
                            lhsT=h_all[:, m, j * 128:(j + 1) * 128],
                            rhs=rhs, start=(m == 0), stop=False)
                for j in range(NJ):
                    nc.tensor.matmul(
                        bps[j][:], lhsT=ones1b[:],
                        rhs=b2_sb[:, g * 512:(g + 1) * 512],
                        start=False, stop=True)
                    # drain + gate-scale into bf16 exchange buffer
                    bsb = pbs.tile([128, 512], BF16, tag="bsb")
                    nc.vector.tensor_scalar(
                        out=bsb[:], in0=bps[j][:], scalar1=gate_s[j][:, 2:3],
                        scalar2=None, op0=mybir.AluOpType.mult)
                    nc.scalar.dma_start(
                        ex_in.ap()[g, j * 128:(j + 1) * 128, :], bsb[:])
                # exchange this chunk while the next one computes
                nc.gpsimd.collective_compute(
                    "AllGather", mybir.AluOpType.bypass, replica_groups=pairs,
                    ins=[ex_in.ap()[g]], outs=[ex_out.ap()[g]],
                )
                for j in range(NJ):
                    a0 = pfa.tile([128, 512], BF16, tag="a0")
                    nc.scalar.dma_start(
                        a0[:], ex_out.ap()[g, 0, j * 128:(j + 1) * 128, :])
                    a1 = pfa.tile([128, 512], BF16, tag="a1")
                    nc.scalar.dma_start(
                        a1[:], ex_out.ap()[g, 1, j * 128:(j + 1) * 128, :])
                    art = pfa.tile([128, 512], F32, tag="art")
                    nc.vector.tensor_tensor(out=art[:], in0=a0[:], in1=a1[:],
                                            op=mybir.AluOpType.add)
                    sc = nc.gpsimd.indirect_dma_start(
                        out=out.ap(),
                        out_offset=IndirectOffsetOnAxis(
                            ap=dest_i[j][:, 0:1], axis=0),
                        in_=art[:], in_offset=None,
                        element_offset=g * 512,
                        bounds_check=HALF - 1, oob_is_err=False,
                    )
                    for r in residual_dmas:
                        add_dep_helper(sc.ins, r.ins, sync=True,
                                       reason="scatter after residual copy")

    return nc


# ---------------------------------------------------------------------------
# Host-side wrapper
# ---------------------------------------------------------------------------

_BUILT = {}


def _get_nc(S, D, DFF, K):
    key = (S, D, DFF, K)
    if key not in _BUILT:
        from concourse import bacc
        nc = bacc.Bacc(trn_type="TRN2", num_devices=NC_CORES, debug=False)
        build_mod_kernel(nc, S, D, DFF, K)
        nc.compile()
        _BUILT[key] = nc
    return _BUILT[key]


def make_in_maps(x, W_r, b_r, W1, b1, W2, b2, S, D, DFF, K):
    import ml_dtypes
    bf = ml_dtypes.bfloat16
    HALF = S // 2
    DFFH = DFF // 2
    in_maps = []
    ND = D // 128
    NM = DFFH // 128
    w1sh, w2sh, b1sh = [], [], []
    for h in range(2):
        w1s = np.ascontiguousarray(W1[:, h * DFFH:(h + 1) * DFFH]).astype(bf)
        # blocks [mp, k, 128, 1024]
        w1sh.append(np.ascontiguousarray(
            w1s.reshape(ND, 128, NM // 8, 1024).transpose(2, 0, 1, 3)))
        w2s = np.ascontiguousarray(W2[h * DFFH:(h + 1) * DFFH, :]).astype(bf)
        # blocks [gp, m, 128, 1024]
        w2sh.append(np.ascontiguousarray(
            w2s.reshape(NM, 128, D // 1024, 1024).transpose(2, 0, 1, 3)))
        # b1 pre-transposed to [128, NM]
        b1sh.append(np.ascontiguousarray(
            b1[h * DFFH:(h + 1) * DFFH].reshape(NM, 128).T.astype(np.float32)))
    b2half = (0.5 * b2).astype(bf).reshape(1, D)
    xbf = [np.ascontiguousarray(x[b]).astype(bf) for b in range(x.shape[0])]
    for c in range(NC_CORES):
        b, h = c // 2, c % 2
        in_maps.append({
            "x_own": np.ascontiguousarray(x[b, h * HALF:(h + 1) * HALF, :]),
            "x_bf": xbf[b],
            "wr": W_r.reshape(1, D).astype(np.float32),
            "br": b_r.reshape(1, 1).astype(np.float32),
            "w1": w1sh[h],
            "w2": w2sh[h],
            "b1s": b1sh[h].astype(np.float32),
            "b2h": b2half,
            "hoff": np.array([[h * HALF]], dtype=np.float32),
        })
    return in_maps


def kernel(x, W_r, b_r, W1, b1, W2, b2, position_ids=None, cache_position=None,
           **unused):
    x = np.asarray(x, dtype=np.float32)
    W_r = np.asarray(W_r, dtype=np.float32)
    b_r = np.asarray(b_r, dtype=np.float32)
    W1 = np.asarray(W1, dtype=np.float32)
    b1 = np.asarray(b1, dtype=np.float32)
    W2 = np.asarray(W2, dtype=np.float32)
    b2 = np.asarray(b2, dtype=np.float32)
    B, S, D = x.shape
    DFF = W1.shape[1]
    K = 512
    HALF = S // 2
    nc = _get_nc(S, D, DFF, K)
    in_maps = make_in_maps(x, W_r, b_r, W1, b1, W2, b2, S, D, DFF, K)
    res = run_bass_kernel_spmd(nc, in_maps, list(range(NC_CORES)))
    out = np.empty((B, S, D), dtype=np.float32)
    for c in range(NC_CORES):
        b, h = c // 2, c % 2
        out[b, h * HALF:(h + 1) * HALF, :] = res.results[c]["out"]
    return out


# revision 15
# speedup vs baseline: 1.2057x; 1.2057x over previous
"""Trainium2 Bass kernel for MoD (mixture-of-depths) routing FFN.

Semantics (matching the reference):
  w = x @ W_r + b_r                        # [B, S] router weights
  t_b = K-th largest of w[b, :]            # per-row threshold (K=512)
  selected: w > t_b (strict; ties at threshold dropped)
  out[b, s] = w[b,s] * (gelu(x[b,s] @ W1 + b1) @ W2 + b2)   if selected
  out[b, s] = x[b, s]                                        otherwise

Sharding: 8 cores; cores (2b, 2b+1) form a pair handling batch row b.
Each core routes half the row; the pair AllGathers router weights; exact
top-K selection runs via a 128-bin histogram (exact fp32 compares only
against values inside the bin containing the K-th value, compacted by a
matmul); selected tokens are compacted into K slots via matmul-based
stream compaction; the FFN runs tensor-parallel over the pair (W1
column-split / W2 row-split, bf16 compute); partial outputs are
gate-scaled, exchanged pairwise via bf16 AllGather chunks and summed
locally before the scatter. Routing, selection and the residual path
stay fully fp32.

All one-hot / comparison builds are single wide tensor_tensor ops over
stride-0 broadcast APs (per-partition-scalar tensor_scalar ops have
~1.2us fixed cost each on DVE); partition reductions ride tiny matmuls.
"""

from contextlib import ExitStack

import numpy as np

import concourse.bass as bass
import concourse.tile as tile
from concourse import mybir
from concourse.bass import IndirectOffsetOnAxis
from concourse.bass_utils import run_bass_kernel_spmd
from concourse.masks import make_identity
from concourse.tile_rust import add_dep_helper

F32 = mybir.dt.float32
BF16 = mybir.dt.bfloat16
FP16 = mybir.dt.float16
I32 = mybir.dt.int32

NC_CORES = 8

NBINS = 128          # histogram bins over w in [-4, 4]
BIN_LO = -4.0
BIN_SCALE = NBINS / 8.0


def build_mod_kernel(nc, S, D, DFF, K):
    """Emit the per-core SPMD program. Pair = (2b, 2b+1) handles row b."""
    HALF = S // 2
    DFFH = DFF // 2
    CAP = K                      # slots per row (max selected = K-1 < CAP)
    KT = HALF // 128             # x tiles per core (16)
    TT = S // 128                # token tiles per row (32)
    NJ = CAP // 128              # slot tiles (4)
    ND = D // 128                # d 128-tiles (16)
    NM = DFFH // 128             # dff-col tiles (32)
    NMP = NM // 8                # mm1 mp-groups of 8 m-tiles (4)
    NGRP = D // 512              # mm2 d-chunks == exchange chunks (4)

    x_own = nc.declare_dram_parameter("x_own", [HALF, D], F32, isOutput=False)
    x_bf = nc.declare_dram_parameter("x_bf", [S, D], BF16, isOutput=False)
    wr = nc.declare_dram_parameter("wr", [1, D], F32, isOutput=False)
    br = nc.declare_dram_parameter("br", [1, 1], F32, isOutput=False)
    w1 = nc.declare_dram_parameter("w1", [NMP, ND, 128, 1024], BF16,
                                   isOutput=False)
    w2 = nc.declare_dram_parameter("w2", [2, NM, 128, 1024], BF16,
                                   isOutput=False)
    b1s = nc.declare_dram_parameter("b1s", [128, NM], F32, isOutput=False)
    b2h = nc.declare_dram_parameter("b2h", [1, D], BF16, isOutput=False)
    hoff = nc.declare_dram_parameter("hoff", [1, 1], F32, isOutput=False)
    out = nc.declare_dram_parameter("out", [HALF, D], F32, isOutput=True)

    # Internal DRAM for collectives (pair groups).
    ag_in = nc.dram_tensor("ag_in", [1, HALF], F32)
    ag_out = nc.dram_tensor("ag_out", [2, HALF], F32)
    ex_in = nc.dram_tensor("ex_in", [NGRP, CAP, 512], BF16)
    ex_out = nc.dram_tensor("ex_out", [NGRP, 2, CAP, 512], BF16)
    pairs = [[2 * b, 2 * b + 1] for b in range(NC_CORES // 2)]

    AOT = mybir.AluOpType

    with tile.TileContext(nc) as tc, ExitStack() as ctx:
        pc = ctx.enter_context(tc.tile_pool(name="const", bufs=1))
        pr = ctx.enter_context(tc.tile_pool(name="route", bufs=1))
        # big iota tables, freed after the compact phase
        _pio_cm = tc.tile_pool(name="pio", bufs=1)
        pio = _pio_cm.__enter__()

        # ---- small input broadcasts (Pool queue; keep this list short) ----
        wr1 = pc.tile([1, D], F32, name="wr1")
        nc.sync.dma_start(wr1[:], wr.ap())
        wr_bc = pc.tile([128, D], F32, name="wr_bc")
        nc.gpsimd.partition_broadcast(wr_bc[:], wr1[:], 128)
        br1 = pc.tile([1, 1], F32, name="br1")
        nc.sync.dma_start(br1[:], br.ap())
        br_bc = pc.tile([128, 1], F32, name="br_bc")
        nc.gpsimd.partition_broadcast(br_bc[:], br1[:], 128)
        ho1 = pc.tile([1, 1], F32, name="ho1")
        nc.sync.dma_start(ho1[:], hoff.ap())
        ho_bc = pc.tile([128, 1], F32, name="ho_bc")
        nc.gpsimd.partition_broadcast(ho_bc[:], ho1[:], 128)
        b1_sb = pc.tile([128, NM], F32, name="b1_sb")
        nc.scalar.dma_start(b1_sb[:], b1s.ap())
        b2_sb = pc.tile([1, D], BF16, name="b2_sb")
        nc.scalar.dma_start(b2_sb[:], b2h.ap())

        # ---- constants ----
        ident = pc.tile([128, 128], F32, name="ident")
        make_identity(nc, ident[:])
        ident_b = pc.tile([128, 128], BF16, name="ident_b")
        nc.vector.tensor_copy(ident_b[:], ident[:])
        ones128 = pc.tile([128, 1], F32, name="ones128")
        nc.vector.memset(ones128[:], 1.0)
        ones1b = pc.tile([1, 128], BF16, name="ones1b")
        nc.vector.memset(ones1b[:], 1.0)
        onesb_col = pc.tile([128, 1], BF16, name="onesb_col")
        nc.vector.memset(onesb_col[:], 1.0)
        # U strict-upper triangulars (as stored): U[q, p] = 1 iff q < p
        uTT = pc.tile([TT, TT], F32, name="uTT")
        nc.gpsimd.memset(uTT[:], 0.0)
        nc.gpsimd.affine_select(
            out=uTT[:], in_=uTT[:], compare_op=AOT.is_ge,
            fill=1.0, base=0, pattern=[[-1, TT]], channel_multiplier=1,
        )
        u128 = pc.tile([128, 128], F32, name="u128")
        nc.gpsimd.memset(u128[:], 0.0)
        nc.gpsimd.affine_select(
            out=u128[:], in_=u128[:], compare_op=AOT.is_ge,
            fill=1.0, base=0, pattern=[[-1, 128]], channel_multiplier=1,
        )
        # iota over partitions [128, 1]: value = p
        iota_p = pc.tile([128, 1], F32, name="iota_p")
        nc.gpsimd.iota(iota_p[:], pattern=[[0, 1]], base=0,
                       channel_multiplier=1, allow_small_or_imprecise_dtypes=True)
        # iota_cb[p, (c, b)] = b, for bins / bin-compact one-hots (f32)
        iota_cb = pio.tile([128, TT * NBINS], F32, name="iota_cb")
        nc.gpsimd.iota(iota_cb[:].rearrange("p (c b) -> p c b", b=NBINS),
                       pattern=[[0, TT], [1, NBINS]], base=0,
                       channel_multiplier=0, allow_small_or_imprecise_dtypes=True)
        # iota_cs[p, (c, s)] = s, fp16, for the slot one-hot (half of TT)
        iota_cs = pio.tile([128, (TT // 2) * CAP], FP16, name="iota_cs")
        nc.gpsimd.iota(iota_cs[:].rearrange("p (c s) -> p c s", s=CAP),
                       pattern=[[0, TT // 2], [1, CAP]], base=0,
                       channel_multiplier=0, allow_small_or_imprecise_dtypes=True)
        # compact lhsT rows, fp16-exact: [p+1, c, gate] per token column c
        tg3 = pc.tile([128, 3 * TT], FP16, name="tg3")
        tg3v = tg3[:].rearrange("p (c three) -> p c three", three=3)
        nc.gpsimd.iota(tg3v[:, :, 0], pattern=[[0, TT]], base=1,
                       channel_multiplier=1, allow_small_or_imprecise_dtypes=True)
        nc.gpsimd.iota(tg3v[:, :, 1], pattern=[[1, TT]], base=0,
                       channel_multiplier=0, allow_small_or_imprecise_dtypes=True)

        # ---- phase R: router dot ----
        w_mine = pr.tile([128, KT], F32, name="w_mine")
        with tc.tile_pool(name="xs", bufs=8) as px, \
             tc.tile_pool(name="jr", bufs=1) as pjr:
            for k in range(KT):
                xt = px.tile([128, D], F32)
                nc.sync.dma_start(xt[:], x_own.ap()[k * 128:(k + 1) * 128, :])
                jt = pjr.tile([128, D], F32, tag="jR")
                nc.vector.scalar_tensor_tensor(
                    out=jt[:], in0=xt[:], scalar=1.0, in1=wr_bc[:],
                    op0=AOT.bypass, op1=AOT.mult,
                    accum_out=w_mine[:, k:k + 1],
                )
            w_full = pr.tile([128, KT], F32, name="w_full")
            nc.vector.tensor_scalar_add(w_full[:], w_mine[:], br_bc[:, 0:1])
            # transpose to [KT, 128] so the DRAM write (l = k*128 + p) is
            # contiguous instead of a 4-byte-packet strided DMA
            with tc.tile_pool(name="pwt", bufs=1, space="PSUM") as pwt:
                wfT_ps = pwt.tile([KT, 128], F32, name="wfT_ps")
                nc.tensor.transpose(wfT_ps[:], w_full[:], ident[:])
                wfT = pr.tile([KT, 128], F32, name="wfT")
                nc.vector.tensor_copy(wfT[:], wfT_ps[:])
            nc.scalar.dma_start(
                ag_in.ap().rearrange("o (k p) -> (o k) p", p=128), wfT[:])

        # ---- AllGather router weights within pair ----
        nc.gpsimd.collective_compute(
            "AllGather", AOT.bypass, replica_groups=pairs,
            ins=[ag_in.ap()], outs=[ag_out.ap()],
        )

        # ---- phase RANK: exact top-K via 128-bin histogram ----
        # w_tok[p, c] = w[c*128 + p] over the full row (rank order in
        # ag_out is [first half, second half] on both cores); w_bc[j, t]
        # = w_t on every partition j.
        _prank_cm = tc.tile_pool(name="prank", bufs=1)
        prank = _prank_cm.__enter__()
        w_tok = pr.tile([128, TT], F32, name="w_tok")
        wrow = prank.tile([1, S], F32, name="wrow")
        nc.scalar.dma_start(wrow[:, 0:HALF], ag_out.ap()[0:1, :])
        nc.scalar.dma_start(wrow[:, HALF:S], ag_out.ap()[1:2, :])
        w_bc = prank.tile([128, S], F32, name="w_bc")
        nc.gpsimd.partition_broadcast(w_bc[:], wrow[:], 128)
        with tc.tile_pool(name="pagt", bufs=2) as pagt, \
             tc.tile_pool(name="pwk", bufs=2, space="PSUM") as pwk:
            for h in range(2):
                agh = pagt.tile([KT, 128], F32, tag="agh")
                nc.scalar.dma_start(
                    agh[:],
                    ag_out.ap()[h:h + 1, :].rearrange(
                        "o (k p) -> (o k) p", p=128))
                wt_ps = pwk.tile([128, KT], F32, tag="wtps")
                nc.tensor.transpose(wt_ps[:], agh[:], ident[0:KT, 0:KT])
                nc.vector.tensor_copy(w_tok[:, h * KT:(h + 1) * KT], wt_ps[:])

        # bins: bin(t) = clamp(int((w+4)*16), 0, 127), exact small ints
        binf = pr.tile([128, TT], F32, name="binf")
        nc.vector.tensor_scalar(
            out=binf[:], in0=w_tok[:], scalar1=-BIN_LO, scalar2=BIN_SCALE,
            op0=AOT.add, op1=AOT.mult)
        nc.vector.tensor_scalar(
            out=binf[:], in0=binf[:], scalar1=0.0, scalar2=float(NBINS - 1),
            op0=AOT.max, op1=AOT.min)
        bin_i = pr.tile([128, TT], I32, name="bin_i")
        nc.vector.tensor_copy(bin_i[:], binf[:])
        bin_f = pr.tile([128, TT], F32, name="bin_f")
        nc.vector.tensor_copy(bin_f[:], bin_i[:])

        _prk_cm = tc.tile_pool(name="prk", bufs=4, space="PSUM")
        prk = _prk_cm.__enter__()
        iota_cb_v = iota_cb[:].rearrange("p (c b) -> p c b", b=NBINS)

        # histogram: one wide one-hot + 32 tiny matmul partition-reduces
        cmpH = prank.tile([128, TT * NBINS], BF16, name="cmpH")
        cmpH_v = cmpH[:].rearrange("p (c b) -> p c b", b=NBINS)
        nc.vector.tensor_tensor(
            out=cmpH_v,
            in0=bin_f[:].rearrange("p c -> p c ()").to_broadcast(
                [128, TT, NBINS]),
            in1=iota_cb_v, op=AOT.is_equal)
        hist_ps = prk.tile([128, 1], F32, tag="rk", name="hist_ps")
        for c in range(TT):
            nc.tensor.matmul(hist_ps[:], lhsT=cmpH_v[:, c, :],
                             rhs=onesb_col[:], start=(c == 0),
                             stop=(c == TT - 1))
        hist = pr.tile([128, 1], F32, name="hist")
        nc.vector.tensor_copy(hist[:], hist_ps[:])
        # nlt[b] = #{t: bin(t) < b};  ngt[b] = S - nlt[b] - hist[b]
        nlt_ps = prk.tile([128, 1], F32, tag="rk", name="nlt_ps")
        nc.tensor.matmul(nlt_ps[:], lhsT=u128[:], rhs=hist[:],
                         start=True, stop=True)
        nlt = pr.tile([128, 1], F32, name="nlt")
        nc.vector.tensor_copy(nlt[:], nlt_ps[:])
        ngt = pr.tile([128, 1], F32, name="ngt")
        nc.vector.scalar_tensor_tensor(
            out=ngt[:], in0=nlt[:], scalar=-1.0, in1=hist[:],
            op0=AOT.mult, op1=AOT.subtract)
        nc.vector.tensor_scalar_add(ngt[:], ngt[:], float(S))
        # crossing bin b*: ngt(b*) <= K-1 and nlt(b*) <= S-K (unique)
        ind1 = pr.tile([128, 1], F32, name="ind1")
        nc.vector.tensor_scalar(out=ind1[:], in0=ngt[:], scalar1=float(K - 1),
                                scalar2=None, op0=AOT.is_le)
        ind = pr.tile([128, 1], F32, name="ind")
        nc.vector.tensor_scalar(out=ind[:], in0=nlt[:], scalar1=float(S - K),
                                scalar2=None, op0=AOT.is_le)
        nc.vector.tensor_tensor(out=ind[:], in0=ind[:], in1=ind1[:],
                                op=AOT.mult)
        # extract [b*, ngt(b*)] and broadcast to partitions
        rhs2 = pr.tile([128, 2], F32, name="rhs2")
        nc.vector.tensor_copy(rhs2[:, 0:1], iota_p[:])
        nc.vector.tensor_copy(rhs2[:, 1:2], ngt[:])
        bst_ps = prk.tile([1, 2], F32, tag="rk", name="bst_ps")
        nc.tensor.matmul(bst_ps[:], lhsT=ind[:], rhs=rhs2[:],
                         start=True, stop=True)
        bst = pr.tile([1, 2], F32, name="bst")
        nc.vector.tensor_copy(bst[:], bst_ps[:])
        bst_bc = pr.tile([128, 2], F32, name="bst_bc")
        nc.gpsimd.partition_broadcast(bst_bc[:], bst[:], 128)

        # above / in-bin masks (stride-0 broadcast of b*)
        above = pr.tile([128, TT], F32, name="above")
        nc.vector.tensor_tensor(
            out=above[:], in0=bin_f[:],
            in1=bst_bc[:, 0:1].to_broadcast([128, TT]), op=AOT.is_gt)
        inbin = pr.tile([128, TT], F32, name="inbin")
        nc.vector.tensor_tensor(
            out=inbin[:], in0=bin_f[:],
            in1=bst_bc[:, 0:1].to_broadcast([128, TT]), op=AOT.is_equal)

        # compact the in-bin w values into wbinT[j, 0] (slot-major column)
        colTm_ps = prk.tile([TT, 1], F32, tag="rk", name="colTm_ps")
        nc.tensor.matmul(colTm_ps[:], lhsT=inbin[:], rhs=ones128[:],
                         start=True, stop=True)
        colTm = pr.tile([TT, 1], F32, name="colTm")
        nc.vector.tensor_copy(colTm[:], colTm_ps[:])
        posm_ps = prk.tile([128, TT], F32, tag="rk", name="posm_ps")
        nc.tensor.matmul(posm_ps[:], lhsT=colTm[:].to_broadcast([TT, 128]),
                         rhs=uTT[:], start=True, stop=False)
        nc.tensor.matmul(posm_ps[:], lhsT=u128[:], rhs=inbin[:],
                         start=False, stop=True)
        posm = pr.tile([128, TT], F32, name="posm")
        nc.vector.tensor_copy(posm[:], posm_ps[:])
        # not-in-bin -> slot 600+pos (never matches a bin slot 0..127)
        posm_m = pr.tile([128, TT], F32, name="posm_m")
        nc.vector.scalar_tensor_tensor(
            out=posm_m[:], in0=inbin[:], scalar=-600.0, in1=posm[:],
            op0=AOT.mult, op1=AOT.add)
        nc.vector.tensor_scalar_add(posm_m[:], posm_m[:], 600.0)
        ohmA = prank.tile([128, TT * NBINS], F32, name="ohmA")
        ohmA_v = ohmA[:].rearrange("p (c b) -> p c b", b=NBINS)
        nc.vector.tensor_tensor(
            out=ohmA_v,
            in0=posm_m[:].rearrange("p c -> p c ()").to_broadcast(
                [128, TT, NBINS]),
            in1=iota_cb_v, op=AOT.is_equal)
        wsel_ps = prk.tile([1, NBINS], F32, tag="rk", name="wsel_ps")
        for c in range(TT):
            nc.tensor.matmul(wsel_ps[:], lhsT=w_tok[:, c:c + 1],
                             rhs=ohmA_v[:, c, :], start=(c == 0),
                             stop=(c == TT - 1))
        wsel = pr.tile([1, NBINS], F32, name="wsel")
        nc.vector.tensor_copy(wsel[:], wsel_ps[:])
        wbinT_ps = prk.tile([128, 1], F32, tag="rk", name="wbinT_ps")
        nc.tensor.transpose(wbinT_ps[:], wsel[:], ident[0:1, 0:1])
        wbinT = pr.tile([128, 1], F32, name="wbinT")
        nc.vector.tensor_copy(wbinT[:], wbinT_ps[:])

        # counts_in(t) = #{j in b*: w_j >= w_t}, exact fp32 compares:
        # cmpC[j, t] = [wbin_j >= w_t], then per-column partition reduce.
        # (pad slots are 0.0 < theta, so they contribute 0 for t in b*)
        cmpC = prank.tile([128, S], BF16, name="cmpC")
        nc.vector.tensor_tensor(
            out=cmpC[:], in0=wbinT[:, 0:1].to_broadcast([128, S]),
            in1=w_bc[:], op=AOT.is_ge)
        cin_ps = prk.tile([128, TT], F32, tag="rk", name="cin_ps")
        for c in range(TT):
            nc.tensor.matmul(cin_ps[:, c:c + 1],
                             lhsT=cmpC[:, c * 128:(c + 1) * 128],
                             rhs=onesb_col[:], start=True, stop=True)
        cin = pr.tile([128, TT], F32, name="cin")
        nc.vector.tensor_copy(cin[:], cin_ps[:])

        # selected = above OR (inbin AND cin <= K-1-ngt(b*))
        thr = pr.tile([128, 1], F32, name="thr")
        nc.vector.tensor_scalar(out=thr[:], in0=bst_bc[:, 1:2], scalar1=-1.0,
                                scalar2=float(K - 1), op0=AOT.mult,
                                op1=AOT.add)
        sel = pr.tile([128, TT], F32, name="sel")
        nc.vector.tensor_tensor(out=sel[:], in0=cin[:],
                                in1=thr[:, 0:1].to_broadcast([128, TT]),
                                op=AOT.is_le)
        nc.vector.tensor_tensor(out=sel[:], in0=sel[:], in1=inbin[:],
                                op=AOT.mult)
        nc.vector.tensor_tensor(out=sel[:], in0=sel[:], in1=above[:],
                                op=AOT.add)
        unsel = pr.tile([128, TT], F32, name="unsel")
        nc.vector.tensor_scalar(out=unsel[:], in0=sel[:], scalar1=-1.0,
                                scalar2=1.0, op0=AOT.mult, op1=AOT.add)
        gate = pr.tile([128, TT], F32, name="gate")
        nc.vector.tensor_tensor(out=gate[:], in0=sel[:], in1=w_tok[:],
                                op=AOT.mult)
        nc.vector.tensor_copy(tg3v[:, :, 2], gate[:])

        # ---- phase PREFIX: exclusive prefix-sum of sel over t = c*128+p ----
        colT_ps = prk.tile([TT, 1], F32, tag="rk", name="colT_ps")
        nc.tensor.matmul(colT_ps[:], lhsT=sel[:], rhs=ones128[:],
                         start=True, stop=True)
        colT = pr.tile([TT, 1], F32, name="colT")
        nc.vector.tensor_copy(colT[:], colT_ps[:])
        pos_ps = prk.tile([128, TT], F32, tag="rk", name="pos_ps")
        nc.tensor.matmul(pos_ps[:], lhsT=colT[:].to_broadcast([TT, 128]),
                         rhs=uTT[:], start=True, stop=False)
        nc.tensor.matmul(pos_ps[:], lhsT=u128[:], rhs=sel[:],
                         start=False, stop=True)
        pos = pr.tile([128, TT], F32, name="pos")
        nc.vector.tensor_copy(pos[:], pos_ps[:])
        # unselected -> slot 600+pos (<= 1111, exact in fp16, > 511)
        pos_m = pr.tile([128, TT], F32, name="pos_m")
        nc.vector.scalar_tensor_tensor(
            out=pos_m[:], in0=unsel[:], scalar=600.0, in1=pos[:],
            op0=AOT.mult, op1=AOT.add)
        pos_m16 = pr.tile([128, TT], FP16, name="pos_m16")
        nc.vector.tensor_copy(pos_m16[:], pos_m[:])

        # ---- phase COMPACT: slot -> (p+1, c, gate) via fp16 matmuls ----
        tok_i = []   # int32 gather offsets per slot tile
        gate_s = []  # f32 per-slot gates
        dest_i = []  # int32 scatter offsets (OOB for pad/other-half)
        TH = TT // 2
        cps = prk.tile([3, CAP], F32, tag="rk", name="cps")
        for half in range(2):
            mtA = prank.tile([128, TH * CAP], FP16, tag="mtA", name="mtA")
            mtA_v = mtA[:].rearrange("p (c s) -> p c s", s=CAP)
            nc.vector.tensor_tensor(
                out=mtA_v,
                in0=pos_m16[:, half * TH:(half + 1) * TH].rearrange(
                    "p c -> p c ()").to_broadcast([128, TH, CAP]),
                in1=iota_cs[:].rearrange("p (c s) -> p c s", s=CAP),
                op=AOT.is_equal)
            for ch in range(TH):
                c = half * TH + ch
                nc.tensor.matmul(cps[:], lhsT=tg3[:, 3 * c:3 * c + 3],
                                 rhs=mtA_v[:, ch, :],
                                 start=(c == 0), stop=(c == TT - 1))
        compact = pr.tile([3, CAP], F32, name="compact")
        nc.vector.tensor_copy(compact[:], cps[:])
        # batched per-slot index math on [128, NJ] tiles
        cpjs = pr.tile([128, NJ, 3], F32, name="cpjs")
        with tc.tile_pool(name="ptp", bufs=4, space="PSUM") as ptp:
            for j in range(NJ):
                tp = ptp.tile([128, 3], F32, tag="tp")
                nc.tensor.transpose(tp[:], compact[:, j * 128:(j + 1) * 128],
                                    ident[0:3, 0:3])
                nc.vector.tensor_copy(cpjs[:, j, :], tp[:])
                cpj = pr.tile([128, 3], F32, name=f"cpj{j}")
                nc.vector.tensor_copy(cpj[:], tp[:])
                gate_s.append(cpj)
        # tokp1 = 128*c + (p+1) == token id + 1; 0 for pad slots
        tokp1 = pr.tile([128, NJ], F32, name="tokp1")
        nc.vector.scalar_tensor_tensor(
            out=tokp1[:], in0=cpjs[:, :, 1], scalar=128.0, in1=cpjs[:, :, 0],
            op0=AOT.mult, op1=AOT.add)
        tif = pr.tile([128, NJ], F32, name="tif")
        nc.vector.tensor_scalar(out=tif[:], in0=tokp1[:], scalar1=-1.0,
                                scalar2=0.0, op0=AOT.add, op1=AOT.max)
        tii = pr.tile([128, NJ], I32, name="tii")
        nc.vector.tensor_copy(tii[:], tif[:])
        for j in range(NJ):
            tok_i.append(tii[:, j:j + 1])
        # scatter offset: (tokp1 - 1) - hoff; OOB for pad/other-half
        df = pr.tile([128, NJ], F32, name="df")
        nc.vector.scalar_tensor_tensor(
            out=df[:], in0=tokp1[:], scalar=-1.0,
            in1=ho_bc[:, 0:1].to_broadcast([128, NJ]),
            op0=AOT.add, op1=AOT.subtract)
        okm = pr.tile([128, NJ], F32, name="okm")
        nc.vector.tensor_scalar(out=okm[:], in0=df[:], scalar1=0.0,
                                scalar2=None, op0=AOT.is_ge)
        ok2 = pr.tile([128, NJ], F32, name="ok2")
        nc.vector.tensor_scalar(out=ok2[:], in0=df[:],
                                scalar1=float(HALF - 1), scalar2=None,
                                op0=AOT.is_le)
        nc.vector.tensor_tensor(out=okm[:], in0=okm[:], in1=ok2[:],
                                op=AOT.mult)
        # dfm = okm * (df - BIG) + BIG  (df when ok, BIG when not)
        BIG = float(8 * HALF + 11)
        dfm = pr.tile([128, NJ], F32, name="dfm")
        nc.vector.tensor_scalar_add(dfm[:], df[:], -BIG)
        nc.vector.tensor_tensor(out=dfm[:], in0=dfm[:], in1=okm[:],
                                op=AOT.mult)
        nc.vector.tensor_scalar_add(dfm[:], dfm[:], BIG)
        dii = pr.tile([128, NJ], I32, name="dii")
        nc.vector.tensor_copy(dii[:], dfm[:])
        for j in range(NJ):
            dest_i.append(dii[:, j:j + 1])

        _prk_cm.__exit__(None, None, None)
        _prank_cm.__exit__(None, None, None)
        _pio_cm.__exit__(None, None, None)

        # ---- phase GATHER: xg rows (bf16) -> transpose -> xgT ----
        xgT = pr.tile([128, ND, CAP], BF16, name="xgT")
        with tc.tile_pool(name="pxg", bufs=2) as pxg, \
             tc.tile_pool(name="ptg", bufs=4, space="PSUM") as ptg:
            for j in range(NJ):
                xg = pxg.tile([128, D], BF16, tag="xg")
                nc.gpsimd.indirect_dma_start(
                    out=xg[:], out_offset=None, in_=x_bf.ap(),
                    in_offset=IndirectOffsetOnAxis(ap=tok_i[j], axis=0),
                )
                for kq in range(ND // 4):
                    tps = ptg.tile([128, 512], BF16, tag="tps")
                    for q in range(4):
                        k = kq * 4 + q
                        nc.tensor.transpose(
                            tps[:, q * 128:(q + 1) * 128],
                            xg[:, k * 128:(k + 1) * 128], ident_b[:])
                    if kq % 2 == 0:
                        for q in range(4):
                            k = kq * 4 + q
                            nc.vector.tensor_copy(
                                xgT[:, k, j * 128:(j + 1) * 128],
                                tps[:, q * 128:(q + 1) * 128])
                    else:
                        for q in range(4):
                            k = kq * 4 + q
                            nc.scalar.activation(
                                out=xgT[:, k, j * 128:(j + 1) * 128],
                                in_=tps[:, q * 128:(q + 1) * 128],
                                func=mybir.ActivationFunctionType.Copy,
                                scale=1.0)

        # ---- deferred residual copy: out = x (DRAM->DRAM) on the SWDGE
        # queue so it never blocks the W1/W2 streams on the SP queue ----
        residual_dmas = []
        for k in range(KT // 4):
            r = nc.gpsimd.dma_start(
                out.ap()[k * 512:(k + 1) * 512, :],
                x_own.ap()[k * 512:(k + 1) * 512, :])
            residual_dmas.append(r)

        # ---- phase MM1 + gelu: h[dffcol, toks] = gelu(xg @ W1 + b1) ----
        h_all = pr.tile([128, NM, CAP], BF16, name="h_all")
        with tc.tile_pool(name="pw1", bufs=36) as pw1, \
             tc.tile_pool(name="ph1", bufs=1, space="PSUM") as ph1:
            for mp in range(NMP):
                hps = [ph1.tile([128, CAP], F32, tag=f"hp{i}", name=f"hp{i}")
                       for i in range(8)]
                for k in range(ND):
                    w1c = pw1.tile([128, 1024], BF16, tag="w1c")
                    nc.sync.dma_start(w1c[:], w1.ap()[mp, k])
                    for i in range(8):
                        nc.tensor.matmul(
                            hps[i][:], lhsT=w1c[:, i * 128:(i + 1) * 128],
                            rhs=xgT[:, k, :], start=(k == 0), stop=(k == ND - 1))
                for i in range(8):
                    m = mp * 8 + i
                    nc.scalar.activation(
                        out=h_all[:, m, :], in_=hps[i][:],
                        func=mybir.ActivationFunctionType.Gelu_apprx_tanh,
                        bias=b1_sb[:, m:m + 1], scale=1.0)

        # ---- phase MM2 + pair exchange + combine/scatter (pipelined) ----
        with tc.tile_pool(name="pw2", bufs=1) as pw2, \
             tc.tile_pool(name="pb2", bufs=2, space="PSUM") as pb2, \
             tc.tile_pool(name="pbs", bufs=6) as pbs, \
             tc.tile_pool(name="pfa", bufs=3) as pfa:
            w2t = [None] * NM

            def finalize(g):
                for j in range(NJ):
                    a0 = pfa.tile([128, 512], BF16, tag="a0", name=f"a0_{g}{j}")
                    nc.scalar.dma_start(
                        a0[:], ex_out.ap()[g, 0, j * 128:(j + 1) * 128, :])
                    a1 = pfa.tile([128, 512], BF16, tag="a1", name=f"a1_{g}{j}")
                    nc.scalar.dma_start(
                        a1[:], ex_out.ap()[g, 1, j * 128:(j + 1) * 128, :])
                    art = pfa.tile([128, 512], F32, tag="art",
                                   name=f"art_{g}{j}")
                    nc.vector.tensor_tensor(out=art[:], in0=a0[:], in1=a1[:],
                                            op=AOT.add)
                    sc = nc.gpsimd.indirect_dma_start(
                        out=out.ap(),
                        out_offset=IndirectOffsetOnAxis(ap=dest_i[j], axis=0),
                        in_=art[:], in_offset=None,
                        element_offset=g * 512,
                        bounds_check=HALF - 1, oob_is_err=False,
                    )
                    for r in residual_dmas:
                        add_dep_helper(sc.ins, r.ins, sync=True,
                                       reason="scatter after residual copy")

            for g in range(NGRP):
                gp, gh = g // 2, g % 2
                bps = [pb2.tile([128, 512], F32, tag=f"bp{j}", name=f"bp{j}")
                       for j in range(NJ)]
                for m in range(NM):
                    if gh == 0:
                        w2t[m] = pw2.tile([128, 1024], BF16, tag=f"w2c{m}",
                                          name=f"w2t{m}")
                        nc.sync.dma_start(w2t[m][:], w2.ap()[gp, m])
                    rhs = w2t[m][:, gh * 512:(gh + 1) * 512]
                    for j in range(NJ):
                        nc.tensor.matmul(
                            bps[j][:],
                            lhsT=h_all[:, m, j * 128:(j + 1) * 128],
                            rhs=rhs, start=(m == 0), stop=False)
                for j in range(NJ):
                    nc.tensor.matmul(
                        bps[j][:], lhsT=ones1b[:],
                        rhs=b2_sb[:, g * 512:(g + 1) * 512],
                        start=False, stop=True)
                    # drain + gate-scale into bf16 exchange buffer
                    bsb = pbs.tile([128, 512], BF16, tag="bsb")
                    nc.vector.tensor_scalar(
                        out=bsb[:], in0=bps[j][:], scalar1=gate_s[j][:, 2:3],
                        scalar2=None, op0=AOT.mult)
                    nc.scalar.dma_start(
                        ex_in.ap()[g, j * 128:(j + 1) * 128, :], bsb[:])
                # exchange this chunk; combine the PREVIOUS chunk while the
                # next chunk computes (keeps the in-order queues flowing)
                nc.gpsimd.collective_compute(
                    "AllGather", AOT.bypass, replica_groups=pairs,
                    ins=[ex_in.ap()[g]], outs=[ex_out.ap()[g]],
                )
                if g > 0:
                    finalize(g - 1)
            finalize(NGRP - 1)

    return nc


# ---------------------------------------------------------------------------
# Host-side wrapper
# ---------------------------------------------------------------------------

_BUILT = {}


def _get_nc(S, D, DFF, K):
    key = (S, D, DFF, K)
    if key not in _BUILT:
        from concourse import bacc
        nc = bacc.Bacc(trn_type="TRN2", num_devices=NC_CORES, debug=False)
        build_mod_kernel(nc, S, D, DFF, K)
        nc.compile()
        _BUILT[key] = nc
    return _BUILT[key]


def make_in_maps(x, W_r, b_r, W1, b1, W2, b2, S, D, DFF, K):
    import ml_dtypes
    bf = ml_dtypes.bfloat16
    HALF = S // 2
    DFFH = DFF // 2
    in_maps = []
    ND = D // 128
    NM = DFFH // 128
    w1sh, w2sh, b1sh = [], [], []
    for h in range(2):
        w1s = np.ascontiguousarray(W1[:, h * DFFH:(h + 1) * DFFH]).astype(bf)
        # blocks [mp, k, 128, 1024]
        w1sh.append(np.ascontiguousarray(
            w1s.reshape(ND, 128, NM // 8, 1024).transpose(2, 0, 1, 3)))
        w2s = np.ascontiguousarray(W2[h * DFFH:(h + 1) * DFFH, :]).astype(bf)
        # blocks [gp, m, 128, 1024]
        w2sh.append(np.ascontiguousarray(
            w2s.reshape(NM, 128, D // 1024, 1024).transpose(2, 0, 1, 3)))
        # b1 pre-transposed to [128, NM]
        b1sh.append(np.ascontiguousarray(
            b1[h * DFFH:(h + 1) * DFFH].reshape(NM, 128).T.astype(np.float32)))
    b2half = (0.5 * b2).astype(bf).reshape(1, D)
    xbf = [np.ascontiguousarray(x[b]).astype(bf) for b in range(x.shape[0])]
    for c in range(NC_CORES):
        b, h = c // 2, c % 2
        in_maps.append({
            "x_own": np.ascontiguousarray(x[b, h * HALF:(h + 1) * HALF, :]),
            "x_bf": xbf[b],
            "wr": W_r.reshape(1, D).astype(np.float32),
            "br": b_r.reshape(1, 1).astype(np.float32),
            "w1": w1sh[h],
            "w2": w2sh[h],
            "b1s": b1sh[h].astype(np.float32),
            "b2h": b2half,
            "hoff": np.array([[h * HALF]], dtype=np.float32),
        })
    return in_maps


def kernel(x, W_r, b_r, W1, b1, W2, b2, position_ids=None, cache_position=None,
           **unused):
    x = np.asarray(x, dtype=np.float32)
    W_r = np.asarray(W_r, dtype=np.float32)
    b_r = np.asarray(b_r, dtype=np.float32)
    W1 = np.asarray(W1, dtype=np.float32)
    b1 = np.asarray(b1, dtype=np.float32)
    W2 = np.asarray(W2, dtype=np.float32)
    b2 = np.asarray(b2, dtype=np.float32)
    B, S, D = x.shape
    DFF = W1.shape[1]
    K = 512
    HALF = S // 2
    nc = _get_nc(S, D, DFF, K)
    in_maps = make_in_maps(x, W_r, b_r, W1, b1, W2, b2, S, D, DFF, K)
    res = run_bass_kernel_spmd(nc, in_maps, list(range(NC_CORES)))
    out = np.empty((B, S, D), dtype=np.float32)
    for c in range(NC_CORES):
        b, h = c // 2, c % 2
        out[b, h * HALF:(h + 1) * HALF, :] = res.results[c]["out"]
    return out
